# revision 8
# baseline (speedup 1.0000x reference)
"""DSFusion kernel for 8x TRN2 NeuronCores.

Computation (per reference):
    out_x = x @ Wx.T + bx ; out_y = y @ Wy.T + by
    sp1 = softplus(out_x) ; sp2 = softplus(out_y)
    alpha_x = sp1 + 1 ; alpha_y = sp2 + 1
    alpha_a = sp1*sp2/C + sp1 + sp2 + 1        (algebraic collapse of the
                                                Dempster-Shafer combination --
                                                all S/b/u/conflict terms cancel)

Sharding: data-parallel over the batch dim, 1024 rows per core; weights and
biases replicated. Host pre-transposes x/y/W so the contraction dim sits on
SBUF partitions and pre-casts matmul operands to bf16 (fp32 PSUM accumulate).

Schedule: row tiles are processed in units of [4,2,1,1]; per unit an X phase
accumulates out_x for all row tiles over 16 K-chunks (2 PSUM half-banks per
row tile, 8 banks for the first unit), spills psum+bias to SBUF, then a Y
phase reuses the same banks. The big first unit amortizes the 4MB weight
stream over 4 row tiles so the DMA ramp (~225 GB/s required) never starves
the PE. All input DMAs are triggered from the SP queue in first-release
order and gated (add_dep_helper) on PE K-chunk anchors ~3 chunks ahead of
use; the ACT queue carries no DMA triggers so epilogues never block behind
gated transfers. The three outputs live in one [P,3,C] tile per row tile,
written in place by the epilogue and shipped with a single DMA (quartered
on the final row tile to keep the exposed tail short).
"""

import numpy as np
import ml_dtypes

BATCH = 8192
DIM = 2048
CLASSES = 1000
NCORES = 8
R = BATCH // NCORES          # rows per core (1024)
P = 128
KCH = DIM // P               # contraction chunks (16)
NKK = KCH // 2               # double chunks (8) -- DMA granularity
NH = CLASSES // 2            # psum half (500, fits one 2KB bank)
RH = R // 2                  # row half for x/y DMA split (512)

_CACHE = {}

# Results of the last device run (for the test harness to inspect timing).
LAST_RESULTS = None


def _split_waits(nc, limit=1):
    """The installed walrus can't lower an instruction carrying more than one
    sync wait. Hoist extra waits onto single-wait NOPs inserted immediately
    before the instruction on the same engine (program order preserves the
    wait-all semantics)."""
    import concourse.mybir as mybir

    for f in nc.m.functions:
        for bb in f.blocks:
            out = []
            changed = False
            for ins in list(bb.instructions):
                si = ins.sync_info
                if si is not None and len(si.on_wait) > limit:
                    waits = list(si.on_wait)
                    extra, keep = waits[:-limit], waits[-limit:]
                    for i, w in enumerate(extra):
                        nop = mybir.InstNoOp(name=f"{ins.name}-ws{i}", ins=[], outs=[])
                        nop.engine = ins.engine
                        nop.sync_info = mybir.SyncInfo(on_wait=[w], on_update=[])
                        out.append(nop)
                    ins.sync_info = mybir.SyncInfo(
                        on_wait=keep, on_update=list(si.on_update)
                    )
                    changed = True
                out.append(ins)
            if changed:
                bb.instructions = out


def _build_nc():
    import concourse.bass as bass
    import concourse.mybir as mybir
    import concourse.tile as tile
    from concourse.vector_clock import ScopedClock, VectorClock

    class LeanTailTileContext(tile.TileContext):
        """Tile's stock tail is drain + two all-engine barriers + sem clears;
        with the single-wait-per-instruction legalization the barrier waits
        explode into a ~10us serial EVSEM parade. Replace with: SP drain
        (single-wait NOPs), a two-semaphore handshake barrier (one wait per
        engine), then gpsimd range-clears everything last."""

        def _drain_and_barrier(self, tick_clock, wait_clock):
            nc = self.nc
            vc = tick_clock.global_clock
            n = len(vc)
            for proc in range(n):
                t = vc[proc]
                if t > 0:
                    nop = nc.sync.nop(nofuse=True, hint=f"tail_wait_{proc}")
                    req = ScopedClock(
                        {None: VectorClock([t if i == proc else 0 for i in range(n)])}
                    )
                    wait_clock.add_sem_waits(nop.ins, req)
            nc.sync.drain()

            semB = nc.alloc_semaphore("tail_barrier_b")
            semC = nc.alloc_semaphore("tail_barrier_c")
            engines = list(nc.engines.values())
            pool_eng = nc.gpsimd
            n_eng = len(engines)
            for e in engines:
                e.nop(nofuse=True, hint="tailb_inc").then_inc(semB, 1)
            for e in engines:
                e.wait_ge(semB, n_eng)
            for e in engines:
                if e is not pool_eng:
                    e.nop(nofuse=True, hint="tailc_inc").then_inc(semC, 1)
            pool_eng.wait_ge(semC, n_eng - 1)

            assert self.sems is not None
            popped = self.nc._tile_sem_poison_stack.pop()
            assert popped is self._sem_poison
            nc.clear_and_free_semaphores(
                list(self.sems.allocated().values()) + [semB, semC]
            )

    dt = mybir.dt

    nc = bass.Bass()

    xT = nc.dram_tensor("xT", [DIM, R], dt.bfloat16, kind="ExternalInput")
    yT = nc.dram_tensor("yT", [DIM, R], dt.bfloat16, kind="ExternalInput")
    wxT = nc.dram_tensor("wxT", [DIM, CLASSES], dt.bfloat16, kind="ExternalInput")
    wyT = nc.dram_tensor("wyT", [DIM, CLASSES], dt.bfloat16, kind="ExternalInput")
    bxb = nc.dram_tensor("bxb", [P, CLASSES], dt.bfloat16, kind="ExternalInput")
    byb = nc.dram_tensor("byb", [P, CLASSES], dt.bfloat16, kind="ExternalInput")

    # combined output: [:, 0] = alpha_a, [:, 1] = alpha_x, [:, 2] = alpha_y
    aall = nc.dram_tensor("alpha_all", [R, 3, CLASSES], dt.float32, kind="ExternalOutput")

    xT3 = xT.rearrange("(ko p) r -> p ko r", p=P)
    yT3 = yT.rearrange("(ko p) r -> p ko r", p=P)
    wxT3 = wxT.rearrange("(ko p) c -> p ko c", p=P)
    wyT3 = wyT.rearrange("(ko p) c -> p ko c", p=P)
    aall3 = aall.rearrange("(t p) x c -> t p x c", p=P)

    # softplus(x) = ln(exp(x) + 1); the installed ACT tables have no direct
    # softplus, but exp and ln share one table set. Pre-activation values are
    # within +-4 so exp cannot overflow.
    EXP = mybir.ActivationFunctionType.Exp
    LN = mybir.ActivationFunctionType.Ln
    ADD = mybir.AluOpType.add
    MULT = mybir.AluOpType.mult

    with LeanTailTileContext(nc) as tc:
        with (
            tc.tile_pool(name="wpool", bufs=1) as wpool,
            tc.tile_pool(name="xpool", bufs=1) as xpool,
            tc.tile_pool(name="epool", bufs=1) as epool,
            tc.tile_pool(name="opool", bufs=1) as opool,
            tc.tile_pool(name="psum", bufs=1, space="PSUM") as ppool,
        ):
            # -- input DMAs ------------------------------------------------
            # All input transfers trigger from the SP sequencer (HWDGE) in
            # first-release order; gates below keep arrivals ~3 K-chunks
            # ahead of PE use without flooding the DMA engines.
            x_sb, y_sb, wx_sb, wy_sb = [], [], [], []
            dmas = []  # (handle, phase, k) ; phase None => ungated

            def dma(t_ap, src, ph, k, eng=None):
                h = (eng or nc.sync).dma_start(t_ap, src)
                dmas.append((h, ph, k))
                return h

            for kk in range(NKK):
                t_ = wpool.tile([P, 2, CLASSES], dt.bfloat16, tag=f"wx{kk}")
                wx_sb.append(t_)
                t_ = xpool.tile([P, 2, R], dt.bfloat16, tag=f"x{kk}")
                x_sb.append(t_)
                t_ = wpool.tile([P, 2, CLASSES], dt.bfloat16, tag=f"wy{kk}")
                wy_sb.append(t_)
                t_ = xpool.tile([P, 2, R], dt.bfloat16, tag=f"y{kk}")
                y_sb.append(t_)

            bx_sb = wpool.tile([P, CLASSES], dt.bfloat16, tag="bx")
            by_sb = wpool.tile([P, CLASSES], dt.bfloat16, tag="by")

            # Sync (SP) queue: x front + wx, then bulk back halves during Y0,
            # then the per-row-tile output DMAs. Queue order == release order
            # (anchors non-decreasing), so no head-of-line blocking.
            # First-touch K-chunks transfer singly so the k=0 matmul gate is
            # only ~380KB.
            # first-touch K-chunks transfer singly: the k=0 matmul gate is
            # only ~380KB and nothing else competes for ramp bandwidth
            dma(x_sb[0][:, 0, 0:RH], xT3[:, 0, 0:RH], None, 0)
            dma(wx_sb[0][:, 0, :], wxT3[:, 0, :], None, 0)
            dma(x_sb[0][:, 1, 0:RH], xT3[:, 1, 0:RH], 0, 0)
            dma(wx_sb[0][:, 1, :], wxT3[:, 1, :], 0, 0)
            for kk in range(1, NKK):
                k = [0, 0, 1, 3, 5, 7, 9, 11][kk]
                dma(x_sb[kk][:, :, 0:RH], xT3[:, 2 * kk:2 * kk + 2, 0:RH], 0, k)
                dma(wx_sb[kk][:], wxT3[:, 2 * kk:2 * kk + 2, :], 0, k)
                if kk == 5:
                    dma(bx_sb[:], bxb[:], 0, 8)
            # bulk x rows 512:1024 during Y0, y rows 512:1024 during X1
            for kk in range(NKK):
                dma(x_sb[kk][:, :, RH:R], xT3[:, 2 * kk:2 * kk + 2, RH:R], 1, 2 * kk)
            for kk in range(NKK):
                dma(y_sb[kk][:, :, RH:R], yT3[:, 2 * kk:2 * kk + 2, RH:R], 2, min(2 * kk, KCH - 1))

            # ACT queue: the y-side front stream, straddling late X0 / early
            # Y0. Gated triggers may block later ACT work, but the X0
            # epilogue's ACT chain (softplus) is only needed by the Y0
            # epilogue ~25us later; the PSUM spills that unblock Y0 matmuls
            # run on DVE and are unaffected.
            for kk in range(NKK):
                kw = (0, 2 * kk + 8) if kk < 4 else (1, 2 * kk - 8)
                ka = (0, 2 * kk + 9) if kk < 3 else (1, 2 * kk - 7)
                dma(wy_sb[kk][:], wyT3[:, 2 * kk:2 * kk + 2, :],
                    kw[0], min(kw[1], KCH - 1), eng=nc.scalar)
                dma(y_sb[kk][:, :, 0:RH], yT3[:, 2 * kk:2 * kk + 2, 0:RH],
                    ka[0], min(ka[1], KCH - 1), eng=nc.scalar)
                if kk == 5:
                    dma(by_sb[:], byb[:], 1, 4, eng=nc.scalar)

            def x_slice(k, r):  # lhsT for global row tile r, K-chunk k
                return x_sb[k // 2][:, k % 2, r * P:(r + 1) * P]

            def y_slice(k, r):
                return y_sb[k // 2][:, k % 2, r * P:(r + 1) * P]

            def wx_slice(k, hs):
                return wx_sb[k // 2][:, k % 2, hs]

            def wy_slice(k, hs):
                return wy_sb[k // 2][:, k % 2, hs]

            HS = [slice(0, NH), slice(NH, CLASSES)]

            # -- compute ---------------------------------------------------
            from concourse.tile_rust import add_dep_helper

            UNITS = [(0, 4), (4, 2), (6, 1), (7, 1)]
            mm_anchor = {}  # (phase_idx, k) -> last MM instruction

            _out_tiles = {}

            def out_tile(r):
                if r not in _out_tiles:
                    _out_tiles[r] = opool.tile(
                        [P, 3, CLASSES], dt.float32, tag=f"out{r % 4}", name=f"out{r % 4}"
                    )
                return _out_tiles[r]

            phase_idx = 0
            for u, (r0, nrt) in enumerate(UNITS):
                # ---- X phase ----
                psx = [
                    [ppool.tile([P, NH], dt.float32, tag=f"ps{jj}_{h}", name=f"ps{jj}_{h}") for h in range(2)]
                    for jj in range(nrt)
                ]
                for k in range(KCH):
                    st, sp = k == 0, k == KCH - 1
                    for jj in range(nrt):
                        lhsT = x_slice(k, r0 + jj)
                        nc.tensor.matmul(psx[jj][0][:], lhsT, wx_slice(k, HS[0]), start=st, stop=sp)
                        mm = nc.tensor.matmul(psx[jj][1][:], lhsT, wx_slice(k, HS[1]), start=st, stop=sp)
                    mm_anchor[(phase_idx, k)] = mm.ins
                phase_idx += 1

                # X epilogue (overlaps the Y phase): bias, softplus, alpha_x
                # into the combined out tile, then w1 = sp1/C + 1 in place.
                t1 = []
                for jj in range(nrt):
                    r = r0 + jj
                    ot = out_tile(r)
                    t_ = epool.tile([P, CLASSES], dt.float32, tag=f"t1_{jj}")
                    nc.vector.tensor_tensor(t_[:, HS[0]], psx[jj][0][:], bx_sb[:, HS[0]], ADD)
                    nc.vector.tensor_tensor(t_[:, HS[1]], psx[jj][1][:], bx_sb[:, HS[1]], ADD)
                    for h in range(2):
                        hs = HS[h]
                        sp1 = t_[:, hs]
                        nc.scalar.activation(sp1, sp1, EXP)
                        nc.scalar.activation(sp1, sp1, LN, bias=1.0)
                        nc.vector.tensor_scalar_add(ot[:, 1, hs], sp1, 1.0)
                        nc.vector.tensor_scalar(sp1, sp1, 1.0 / CLASSES, 1.0, MULT, ADD)
                    t1.append(t_)

                # ---- Y phase ----
                if u == len(UNITS) - 1:
                    # Last row tile: four 250-col quarter groups on separate
                    # PSUM banks; outputs ship per plane as each is ready so
                    # the final drain only waits on a 125KB aa transfer.
                    r = r0
                    QN = CLASSES // 4
                    ot = out_tile(r)
                    nc.sync.dma_start(aall3[r][:, 1, :], ot[:, 1, :])  # alpha_x
                    t_ = epool.tile([P, CLASSES], dt.float32, tag="t2_0")
                    for q in range(4):
                        qs = slice(QN * q, QN * (q + 1))
                        psq = ppool.tile([P, QN], dt.float32, tag=f"ps{q // 2}_{q % 2}", name=f"psq{q}")
                        for k in range(KCH):
                            st, sp = k == 0, k == KCH - 1
                            nc.tensor.matmul(psq[:], y_slice(k, r), wy_slice(k, qs), start=st, stop=sp)
                        sp2 = t_[:, qs]
                        nc.vector.tensor_tensor(sp2, psq[:], by_sb[:, qs], ADD)
                        nc.scalar.activation(sp2, sp2, EXP)
                        nc.scalar.activation(sp2, sp2, LN, bias=1.0)
                        nc.scalar.add(ot[:, 2, qs], sp2, 1.0)
                        nc.sync.dma_start(aall3[r][:, 2, qs], ot[:, 2, qs])  # alpha_y
                        nc.vector.tensor_tensor(sp2, sp2, t1[0][:, qs], MULT)
                        nc.vector.tensor_tensor(ot[:, 0, qs], sp2, ot[:, 1, qs], ADD)
                        nc.sync.dma_start(aall3[r][:, 0, qs], ot[:, 0, qs])  # alpha_a
                    phase_idx += 1
                    continue

                psy = [
                    [ppool.tile([P, NH], dt.float32, tag=f"ps{jj}_{h}", name=f"psy{jj}_{h}") for h in range(2)]
                    for jj in range(nrt)
                ]
                for k in range(KCH):
                    st, sp = k == 0, k == KCH - 1
                    for jj in range(nrt):
                        lhsT = y_slice(k, r0 + jj)
                        nc.tensor.matmul(psy[jj][0][:], lhsT, wy_slice(k, HS[0]), start=st, stop=sp)
                        mm = nc.tensor.matmul(psy[jj][1][:], lhsT, wy_slice(k, HS[1]), start=st, stop=sp)
                    mm_anchor[(phase_idx, k)] = mm.ins
                phase_idx += 1

                # Y epilogue: softplus, alpha_y, alpha_a = sp2*w1 + alpha_x,
                # then one combined output DMA per row tile.
                for jj in range(nrt):
                    r = r0 + jj
                    ot = out_tile(r)
                    t_ = epool.tile([P, CLASSES], dt.float32, tag=f"t2_{jj % 2}")
                    nc.vector.tensor_tensor(t_[:, HS[0]], psy[jj][0][:], by_sb[:, HS[0]], ADD)
                    nc.vector.tensor_tensor(t_[:, HS[1]], psy[jj][1][:], by_sb[:, HS[1]], ADD)
                    for h in range(2):
                        hs = HS[h]
                        sp2 = t_[:, hs]
                        nc.scalar.activation(sp2, sp2, EXP)
                        nc.scalar.activation(sp2, sp2, LN, bias=1.0)
                        nc.scalar.add(ot[:, 2, hs], sp2, 1.0)
                        nc.vector.tensor_tensor(sp2, sp2, t1[jj][:, hs], MULT)
                        nc.vector.tensor_tensor(ot[:, 0, hs], sp2, ot[:, 1, hs], ADD)
                        if u == len(UNITS) - 2:
                            # penultimate unit: ship halves as they finish so
                            # the final drain never waits on a 1.5MB transfer
                            nc.sync.dma_start(aall3[r][:, :, hs], ot[:, :, hs])
                    if u != len(UNITS) - 2:
                        nc.sync.dma_start(aall3[r], ot[:])

            # -- DMA backpressure: gate transfers on PE progress -----------
            for h, ph, k in dmas:
                if ph is not None:
                    add_dep_helper(
                        h.ins, mm_anchor[(ph, max(0, min(k, KCH - 1)))], reason="ramp"
                    )

    _split_waits(nc)
    return nc


def kernel(x, y, Wx, bx, Wy, by):
    global LAST_RESULTS
    from concourse.bass_utils import run_bass_kernel_spmd

    if "nc" not in _CACHE:
        _CACHE["nc"] = _build_nc()
    nc = _CACHE["nc"]

    bf16 = ml_dtypes.bfloat16
    x = np.asarray(x, dtype=np.float32)
    y = np.asarray(y, dtype=np.float32)
    xb = x.astype(bf16)                       # [BATCH, DIM]
    yb = y.astype(bf16)
    wxT = np.ascontiguousarray(np.asarray(Wx, dtype=np.float32).astype(bf16).T)  # [DIM, CLASSES]
    wyT = np.ascontiguousarray(np.asarray(Wy, dtype=np.float32).astype(bf16).T)
    bxb = np.ascontiguousarray(
        np.broadcast_to(np.asarray(bx, dtype=np.float32).astype(bf16), (P, CLASSES))
    )
    byb = np.ascontiguousarray(
        np.broadcast_to(np.asarray(by, dtype=np.float32).astype(bf16), (P, CLASSES))
    )

    xTb = np.ascontiguousarray(xb.T)          # [DIM, BATCH]
    yTb = np.ascontiguousarray(yb.T)

    in_maps = []
    for c in range(NCORES):
        rs = slice(c * R, (c + 1) * R)
        in_maps.append(
            {
                "xT": np.ascontiguousarray(xTb[:, rs]),
                "yT": np.ascontiguousarray(yTb[:, rs]),
                "wxT": wxT,
                "wyT": wyT,
                "bxb": bxb,
                "byb": byb,
            }
        )

    res = run_bass_kernel_spmd(nc, in_maps, core_ids=list(range(NCORES)))
    LAST_RESULTS = res

    parts = [res.results[c]["alpha_all"] for c in range(NCORES)]
    aa = np.concatenate([p[:, 0] for p in parts], axis=0)
    ax = np.concatenate([p[:, 1] for p in parts], axis=0)
    ay = np.concatenate([p[:, 2] for p in parts], axis=0)
    return (aa, ax, ay)


# revision 9
# speedup vs baseline: 1.0144x; 1.0144x over previous
"""DSFusion kernel for 8x TRN2 NeuronCores.

Computation (per reference):
    out_x = x @ Wx.T + bx ; out_y = y @ Wy.T + by
    sp1 = softplus(out_x) ; sp2 = softplus(out_y)
    alpha_x = sp1 + 1 ; alpha_y = sp2 + 1
    alpha_a = sp1*sp2/C + sp1 + sp2 + 1        (algebraic collapse of the
                                                Dempster-Shafer combination --
                                                all S/b/u/conflict terms cancel)

Sharding: data-parallel over the batch dim, 1024 rows per core; weights and
biases replicated. Host pre-transposes x/y/W so the contraction dim sits on
SBUF partitions and pre-casts matmul operands to bf16 (fp32 PSUM accumulate).

Schedule: row tiles are processed in units of [4,2,1,1]; per unit an X phase
accumulates out_x for all row tiles over 16 K-chunks (2 PSUM half-banks per
row tile, 8 banks for the first unit), spills psum+bias to SBUF, then a Y
phase reuses the same banks. The big first unit amortizes the 4MB weight
stream over 4 row tiles so the DMA ramp (~225 GB/s required) never starves
the PE. All input DMAs are triggered from the SP queue in first-release
order and gated (add_dep_helper) on PE K-chunk anchors ~3 chunks ahead of
use; the ACT queue carries no DMA triggers so epilogues never block behind
gated transfers. The three outputs live in one [P,3,C] tile per row tile,
written in place by the epilogue and shipped with a single DMA (quartered
on the final row tile to keep the exposed tail short).
"""

import numpy as np
import ml_dtypes

BATCH = 8192
DIM = 2048
CLASSES = 1000
NCORES = 8
R = BATCH // NCORES          # rows per core (1024)
P = 128
KCH = DIM // P               # contraction chunks (16)
NKK = KCH // 2               # double chunks (8) -- DMA granularity
NH = CLASSES // 2            # psum half (500, fits one 2KB bank)
RH = R // 2                  # row half for x/y DMA split (512)

_CACHE = {}

# Results of the last device run (for the test harness to inspect timing).
LAST_RESULTS = None


def _split_waits(nc, limit=1):
    """The installed walrus can't lower an instruction carrying more than one
    sync wait. Hoist extra waits onto single-wait NOPs inserted immediately
    before the instruction on the same engine (program order preserves the
    wait-all semantics)."""
    import concourse.mybir as mybir

    for f in nc.m.functions:
        for bb in f.blocks:
            out = []
            changed = False
            for ins in list(bb.instructions):
                si = ins.sync_info
                if si is not None and len(si.on_wait) > limit:
                    waits = list(si.on_wait)
                    extra, keep = waits[:-limit], waits[-limit:]
                    for i, w in enumerate(extra):
                        nop = mybir.InstNoOp(name=f"{ins.name}-ws{i}", ins=[], outs=[])
                        nop.engine = ins.engine
                        nop.sync_info = mybir.SyncInfo(on_wait=[w], on_update=[])
                        out.append(nop)
                    ins.sync_info = mybir.SyncInfo(
                        on_wait=keep, on_update=list(si.on_update)
                    )
                    changed = True
                out.append(ins)
            if changed:
                bb.instructions = out


def _build_nc():
    import concourse.bass as bass
    import concourse.mybir as mybir
    import concourse.tile as tile
    from concourse.vector_clock import ScopedClock, VectorClock

    class LeanTailTileContext(tile.TileContext):
        """Tile's stock tail is drain + two all-engine barriers + sem clears;
        with the single-wait-per-instruction legalization the barrier waits
        explode into a ~10us serial EVSEM parade. Replace with: SP drain
        (single-wait NOPs), a two-semaphore handshake barrier (one wait per
        engine), then gpsimd range-clears everything last."""

        def _drain_and_barrier(self, tick_clock, wait_clock):
            nc = self.nc
            vc = tick_clock.global_clock
            n = len(vc)
            for proc in range(n):
                t = vc[proc]
                if t > 0:
                    nop = nc.sync.nop(nofuse=True, hint=f"tail_wait_{proc}")
                    req = ScopedClock(
                        {None: VectorClock([t if i == proc else 0 for i in range(n)])}
                    )
                    wait_clock.add_sem_waits(nop.ins, req)
            nc.sync.drain()

            semB = nc.alloc_semaphore("tail_barrier_b")
            semC = nc.alloc_semaphore("tail_barrier_c")
            engines = list(nc.engines.values())
            pool_eng = nc.gpsimd
            n_eng = len(engines)
            for e in engines:
                e.nop(nofuse=True, hint="tailb_inc").then_inc(semB, 1)
            for e in engines:
                e.wait_ge(semB, n_eng)
            for e in engines:
                if e is not pool_eng:
                    e.nop(nofuse=True, hint="tailc_inc").then_inc(semC, 1)
            pool_eng.wait_ge(semC, n_eng - 1)

            assert self.sems is not None
            popped = self.nc._tile_sem_poison_stack.pop()
            assert popped is self._sem_poison
            nc.clear_and_free_semaphores(
                list(self.sems.allocated().values()) + [semB, semC]
            )

    dt = mybir.dt

    nc = bass.Bass()

    xT = nc.dram_tensor("xT", [DIM, R], dt.bfloat16, kind="ExternalInput")
    yT = nc.dram_tensor("yT", [DIM, R], dt.bfloat16, kind="ExternalInput")
    wxT = nc.dram_tensor("wxT", [DIM, CLASSES], dt.bfloat16, kind="ExternalInput")
    wyT = nc.dram_tensor("wyT", [DIM, CLASSES], dt.bfloat16, kind="ExternalInput")
    bxb = nc.dram_tensor("bxb", [P, CLASSES], dt.bfloat16, kind="ExternalInput")
    byb = nc.dram_tensor("byb", [P, CLASSES], dt.bfloat16, kind="ExternalInput")

    # combined output: [:, 0] = alpha_a, [:, 1] = alpha_x, [:, 2] = alpha_y
    aall = nc.dram_tensor("alpha_all", [R, 3, CLASSES], dt.bfloat16, kind="ExternalOutput")

    xT3 = xT.rearrange("(ko p) r -> p ko r", p=P)
    yT3 = yT.rearrange("(ko p) r -> p ko r", p=P)
    wxT3 = wxT.rearrange("(ko p) c -> p ko c", p=P)
    wyT3 = wyT.rearrange("(ko p) c -> p ko c", p=P)
    aall3 = aall.rearrange("(t p) x c -> t p x c", p=P)

    # softplus(x) = ln(exp(x) + 1); the installed ACT tables have no direct
    # softplus, but exp and ln share one table set. Pre-activation values are
    # within +-4 so exp cannot overflow.
    EXP = mybir.ActivationFunctionType.Exp
    LN = mybir.ActivationFunctionType.Ln
    ADD = mybir.AluOpType.add
    MULT = mybir.AluOpType.mult

    with LeanTailTileContext(nc) as tc:
        with (
            tc.tile_pool(name="wpool", bufs=1) as wpool,
            tc.tile_pool(name="xpool", bufs=1) as xpool,
            tc.tile_pool(name="epool", bufs=1) as epool,
            tc.tile_pool(name="opool", bufs=1) as opool,
            tc.tile_pool(name="psum", bufs=1, space="PSUM") as ppool,
        ):
            # -- input DMAs ------------------------------------------------
            # All input transfers trigger from the SP sequencer (HWDGE) in
            # first-release order; gates below keep arrivals ~3 K-chunks
            # ahead of PE use without flooding the DMA engines.
            x_sb, y_sb, wx_sb, wy_sb = [], [], [], []
            dmas = []  # (handle, phase, k) ; phase None => ungated

            def dma(t_ap, src, ph, k, eng=None):
                h = (eng or nc.sync).dma_start(t_ap, src)
                dmas.append((h, ph, k))
                return h

            for kk in range(NKK):
                t_ = wpool.tile([P, 2, CLASSES], dt.bfloat16, tag=f"wx{kk}")
                wx_sb.append(t_)
                t_ = xpool.tile([P, 2, R], dt.bfloat16, tag=f"x{kk}")
                x_sb.append(t_)
                t_ = wpool.tile([P, 2, CLASSES], dt.bfloat16, tag=f"wy{kk}")
                wy_sb.append(t_)
                t_ = xpool.tile([P, 2, R], dt.bfloat16, tag=f"y{kk}")
                y_sb.append(t_)

            bx_sb = wpool.tile([P, CLASSES], dt.bfloat16, tag="bx")
            by_sb = wpool.tile([P, CLASSES], dt.bfloat16, tag="by")

            # Sync (SP) queue: x front + wx, then bulk back halves during Y0,
            # then the per-row-tile output DMAs. Queue order == release order
            # (anchors non-decreasing), so no head-of-line blocking.
            # First-touch K-chunks transfer singly so the k=0 matmul gate is
            # only ~380KB.
            # A single HWDGE queue sustains only ~90-150 GB/s (descriptor
            # fetch serialization), so the X0-era streams alternate between
            # the SP and ACT queues per K-chunk, each queue emitted in
            # release order (anchors monotone => no head-of-line blocking).
            # Gated triggers on ACT may block later ACT work, but the X0
            # epilogue's softplus chain is only needed by the Y0 epilogue
            # ~25us later; the PSUM spills that unblock Y0 matmuls run on
            # DVE and are unaffected.
            sp_q, act_q = [], []  # (release_key, emit_fn)

            def enq(q, key, dst, src):
                q.append((key, dst, src))

            for k in range(KCH):
                key = (0, max(0, k - 3)) if k >= 2 else (None, 0)
                x_dst = x_sb[k // 2][:, k % 2, 0:RH]
                x_src = xT3[:, k, 0:RH]
                w_dst = wx_sb[k // 2][:, k % 2, :]
                w_src = wxT3[:, k, :]
                if k % 2 == 0:
                    enq(sp_q, key, w_dst, w_src); enq(act_q, key, x_dst, x_src)
                else:
                    enq(sp_q, key, x_dst, x_src); enq(act_q, key, w_dst, w_src)
            enq(sp_q, (0, 8), bx_sb[:], bxb[:])
            # y-side front stream straddles late X0 / early Y0
            for k in range(KCH):
                kw = (0, k + 8) if k < 8 else (1, k - 8)
                ka = (0, k + 9) if k < 7 else (1, k - 7)
                kw = (kw[0], min(kw[1], KCH - 1)); ka = (ka[0], min(ka[1], KCH - 1))
                w_dst = wy_sb[k // 2][:, k % 2, :]
                w_src = wyT3[:, k, :]
                y_dst = y_sb[k // 2][:, k % 2, 0:RH]
                y_src = yT3[:, k, 0:RH]
                if k % 2 == 0:
                    enq(sp_q, kw, w_dst, w_src); enq(act_q, ka, y_dst, y_src)
                else:
                    enq(act_q, kw, w_dst, w_src); enq(sp_q, ka, y_dst, y_src)
            enq(act_q, (1, 2), by_sb[:], byb[:])
            # bulk x/y rows 512:1024 (row tiles 4-7) during Y0, on SP
            for kk in range(NKK):
                enq(sp_q, (1, 2 * kk), x_sb[kk][:, :, RH:R], xT3[:, 2 * kk:2 * kk + 2, RH:R])
                enq(sp_q, (1, min(2 * kk + 1, KCH - 1)), y_sb[kk][:, :, RH:R], yT3[:, 2 * kk:2 * kk + 2, RH:R])

            def _key(e):
                (ph, k), _, _ = e
                return (-1, 0) if ph is None else (ph, k)
            for q, eng in ((sp_q, nc.sync), (act_q, nc.scalar)):
                for (ph, k), dst, src in sorted(q, key=_key):
                    dma(dst, src, ph, k, eng=eng)

            def x_slice(k, r):  # lhsT for global row tile r, K-chunk k
                return x_sb[k // 2][:, k % 2, r * P:(r + 1) * P]

            def y_slice(k, r):
                return y_sb[k // 2][:, k % 2, r * P:(r + 1) * P]

            def wx_slice(k, hs):
                return wx_sb[k // 2][:, k % 2, hs]

            def wy_slice(k, hs):
                return wy_sb[k // 2][:, k % 2, hs]

            HS = [slice(0, NH), slice(NH, CLASSES)]

            # -- compute ---------------------------------------------------
            from concourse.tile_rust import add_dep_helper

            UNITS = [(0, 4), (4, 2), (6, 1), (7, 1)]
            mm_anchor = {}  # (phase_idx, k) -> last MM instruction

            _out_tiles = {}

            def out_tile(r):
                if r not in _out_tiles:
                    _out_tiles[r] = opool.tile(
                        [P, 3, CLASSES], dt.bfloat16, tag=f"out{r % 4}", name=f"out{r % 4}"
                    )
                return _out_tiles[r]

            phase_idx = 0
            for u, (r0, nrt) in enumerate(UNITS):
                # ---- X phase ----
                psx = [
                    [ppool.tile([P, NH], dt.float32, tag=f"ps{jj}_{h}", name=f"ps{jj}_{h}") for h in range(2)]
                    for jj in range(nrt)
                ]
                for k in range(KCH):
                    st, sp = k == 0, k == KCH - 1
                    for jj in range(nrt):
                        lhsT = x_slice(k, r0 + jj)
                        nc.tensor.matmul(psx[jj][0][:], lhsT, wx_slice(k, HS[0]), start=st, stop=sp)
                        mm = nc.tensor.matmul(psx[jj][1][:], lhsT, wx_slice(k, HS[1]), start=st, stop=sp)
                    mm_anchor[(phase_idx, k)] = mm.ins
                phase_idx += 1

                # X epilogue (overlaps the Y phase): bias, softplus, alpha_x
                # into the combined out tile, then w1 = sp1/C + 1 in place.
                t1 = []
                for jj in range(nrt):
                    r = r0 + jj
                    ot = out_tile(r)
                    t_ = epool.tile([P, CLASSES], dt.float32, tag=f"t1_{jj}")
                    nc.vector.tensor_tensor(t_[:, HS[0]], psx[jj][0][:], bx_sb[:, HS[0]], ADD)
                    nc.vector.tensor_tensor(t_[:, HS[1]], psx[jj][1][:], bx_sb[:, HS[1]], ADD)
                    for h in range(2):
                        hs = HS[h]
                        sp1 = t_[:, hs]
                        nc.scalar.activation(sp1, sp1, EXP)
                        nc.scalar.activation(sp1, sp1, LN, bias=1.0)
                        nc.vector.tensor_scalar_add(ot[:, 1, hs], sp1, 1.0)
                        nc.vector.tensor_scalar(sp1, sp1, 1.0 / CLASSES, 1.0, MULT, ADD)
                    t1.append(t_)

                # ---- Y phase ----
                if u == len(UNITS) - 1:
                    # Last row tile: four 250-col quarter groups on separate
                    # PSUM banks; outputs ship per plane as each is ready so
                    # the final drain only waits on a 125KB aa transfer.
                    r = r0
                    QN = CLASSES // 4
                    ot = out_tile(r)
                    nc.sync.dma_start(aall3[r][:, 1, :], ot[:, 1, :])  # alpha_x
                    t_ = epool.tile([P, CLASSES], dt.float32, tag="t2_0")
                    for q in range(4):
                        qs = slice(QN * q, QN * (q + 1))
                        psq = ppool.tile([P, QN], dt.float32, tag=f"ps{q // 2}_{q % 2}", name=f"psq{q}")
                        for k in range(KCH):
                            st, sp = k == 0, k == KCH - 1
                            nc.tensor.matmul(psq[:], y_slice(k, r), wy_slice(k, qs), start=st, stop=sp)
                        sp2 = t_[:, qs]
                        nc.vector.tensor_tensor(sp2, psq[:], by_sb[:, qs], ADD)
                        nc.scalar.activation(sp2, sp2, EXP)
                        nc.scalar.activation(sp2, sp2, LN, bias=1.0)
                        nc.scalar.add(ot[:, 2, qs], sp2, 1.0)
                        nc.sync.dma_start(aall3[r][:, 2, qs], ot[:, 2, qs])  # alpha_y
                        nc.vector.tensor_tensor(sp2, sp2, t1[0][:, qs], MULT)
                        nc.vector.tensor_tensor(ot[:, 0, qs], sp2, ot[:, 1, qs], ADD)
                        nc.sync.dma_start(aall3[r][:, 0, qs], ot[:, 0, qs])  # alpha_a
                    phase_idx += 1
                    continue

                psy = [
                    [ppool.tile([P, NH], dt.float32, tag=f"ps{jj}_{h}", name=f"psy{jj}_{h}") for h in range(2)]
                    for jj in range(nrt)
                ]
                for k in range(KCH):
                    st, sp = k == 0, k == KCH - 1
                    for jj in range(nrt):
                        lhsT = y_slice(k, r0 + jj)
                        nc.tensor.matmul(psy[jj][0][:], lhsT, wy_slice(k, HS[0]), start=st, stop=sp)
                        mm = nc.tensor.matmul(psy[jj][1][:], lhsT, wy_slice(k, HS[1]), start=st, stop=sp)
                    mm_anchor[(phase_idx, k)] = mm.ins
                phase_idx += 1

                # Y epilogue: softplus, alpha_y, alpha_a = sp2*w1 + alpha_x,
                # then one combined output DMA per row tile.
                for jj in range(nrt):
                    r = r0 + jj
                    ot = out_tile(r)
                    t_ = epool.tile([P, CLASSES], dt.float32, tag=f"t2_{jj % 2}")
                    nc.vector.tensor_tensor(t_[:, HS[0]], psy[jj][0][:], by_sb[:, HS[0]], ADD)
                    nc.vector.tensor_tensor(t_[:, HS[1]], psy[jj][1][:], by_sb[:, HS[1]], ADD)
                    for h in range(2):
                        hs = HS[h]
                        sp2 = t_[:, hs]
                        nc.scalar.activation(sp2, sp2, EXP)
                        nc.scalar.activation(sp2, sp2, LN, bias=1.0)
                        nc.scalar.add(ot[:, 2, hs], sp2, 1.0)
                        nc.vector.tensor_tensor(sp2, sp2, t1[jj][:, hs], MULT)
                        nc.vector.tensor_tensor(ot[:, 0, hs], sp2, ot[:, 1, hs], ADD)
                        if u == len(UNITS) - 2:
                            # penultimate unit: ship halves as they finish so
                            # the final drain never waits on a 1.5MB transfer
                            nc.sync.dma_start(aall3[r][:, :, hs], ot[:, :, hs])
                    if u != len(UNITS) - 2:
                        nc.sync.dma_start(aall3[r], ot[:])

            # -- DMA backpressure: gate transfers on PE progress -----------
            for h, ph, k in dmas:
                if ph is not None:
                    add_dep_helper(
                        h.ins, mm_anchor[(ph, max(0, min(k, KCH - 1)))], reason="ramp"
                    )

    _split_waits(nc)
    return nc


def kernel(x, y, Wx, bx, Wy, by):
    global LAST_RESULTS
    from concourse.bass_utils import run_bass_kernel_spmd

    if "nc" not in _CACHE:
        _CACHE["nc"] = _build_nc()
    nc = _CACHE["nc"]

    bf16 = ml_dtypes.bfloat16
    x = np.asarray(x, dtype=np.float32)
    y = np.asarray(y, dtype=np.float32)
    xb = x.astype(bf16)                       # [BATCH, DIM]
    yb = y.astype(bf16)
    wxT = np.ascontiguousarray(np.asarray(Wx, dtype=np.float32).astype(bf16).T)  # [DIM, CLASSES]
    wyT = np.ascontiguousarray(np.asarray(Wy, dtype=np.float32).astype(bf16).T)
    bxb = np.ascontiguousarray(
        np.broadcast_to(np.asarray(bx, dtype=np.float32).astype(bf16), (P, CLASSES))
    )
    byb = np.ascontiguousarray(
        np.broadcast_to(np.asarray(by, dtype=np.float32).astype(bf16), (P, CLASSES))
    )

    xTb = np.ascontiguousarray(xb.T)          # [DIM, BATCH]
    yTb = np.ascontiguousarray(yb.T)

    in_maps = []
    for c in range(NCORES):
        rs = slice(c * R, (c + 1) * R)
        in_maps.append(
            {
                "xT": np.ascontiguousarray(xTb[:, rs]),
                "yT": np.ascontiguousarray(yTb[:, rs]),
                "wxT": wxT,
                "wyT": wyT,
                "bxb": bxb,
                "byb": byb,
            }
        )

    res = run_bass_kernel_spmd(nc, in_maps, core_ids=list(range(NCORES)))
    LAST_RESULTS = res

    parts = [res.results[c]["alpha_all"] for c in range(NCORES)]
    aa = np.concatenate([p[:, 0] for p in parts], axis=0).astype(np.float32)
    ax = np.concatenate([p[:, 1] for p in parts], axis=0).astype(np.float32)
    ay = np.concatenate([p[:, 2] for p in parts], axis=0).astype(np.float32)
    return (aa, ax, ay)


# revision 11
# speedup vs baseline: 1.0548x; 1.0398x over previous
"""DSFusion kernel for 8x TRN2 NeuronCores.

Computation (per reference):
    out_x = x @ Wx.T + bx ; out_y = y @ Wy.T + by
    sp1 = softplus(out_x) ; sp2 = softplus(out_y)
    alpha_x = sp1 + 1 ; alpha_y = sp2 + 1
    alpha_a = sp1*sp2/C + sp1 + sp2 + 1        (algebraic collapse of the
                                                Dempster-Shafer combination --
                                                all S/b/u/conflict terms cancel)

Sharding: data-parallel over the batch dim, 1024 rows per core; weights and
biases replicated. Host pre-transposes x/y/W so the contraction dim sits on
SBUF partitions and pre-casts matmul operands to bf16 (fp32 PSUM accumulate).

Schedule: row tiles are processed in units of [4,2,1,1]; per unit an X phase
accumulates out_x for all row tiles over 16 K-chunks (2 PSUM half-banks per
row tile, 8 banks for the first unit), spills psum+bias to SBUF, then a Y
phase reuses the same banks. The big first unit amortizes the 4MB weight
stream over 4 row tiles so the DMA ramp (~225 GB/s required) never starves
the PE. All input DMAs are triggered from the SP queue in first-release
order and gated (add_dep_helper) on PE K-chunk anchors ~3 chunks ahead of
use; the ACT queue carries no DMA triggers so epilogues never block behind
gated transfers. The three outputs live in one [P,3,C] tile per row tile,
written in place by the epilogue and shipped with a single DMA (quartered
on the final row tile to keep the exposed tail short).
"""

import numpy as np
import ml_dtypes

BATCH = 8192
DIM = 2048
CLASSES = 1000
NCORES = 8
R = BATCH // NCORES          # rows per core (1024)
P = 128
KCH = DIM // P               # contraction chunks (16)
NKK = KCH // 2               # double chunks (8) -- DMA granularity
NH = CLASSES // 2            # psum half (500, fits one 2KB bank)
RH = R // 2                  # row half for x/y DMA split (512)

_CACHE = {}

# Results of the last device run (for the test harness to inspect timing).
LAST_RESULTS = None


def _split_waits(nc, limit=1):
    """The installed walrus can't lower an instruction carrying more than one
    sync wait. Hoist extra waits onto single-wait NOPs inserted immediately
    before the instruction on the same engine (program order preserves the
    wait-all semantics)."""
    import concourse.mybir as mybir

    for f in nc.m.functions:
        for bb in f.blocks:
            out = []
            changed = False
            for ins in list(bb.instructions):
                si = ins.sync_info
                if si is not None and len(si.on_wait) > limit:
                    waits = list(si.on_wait)
                    extra, keep = waits[:-limit], waits[-limit:]
                    for i, w in enumerate(extra):
                        nop = mybir.InstNoOp(name=f"{ins.name}-ws{i}", ins=[], outs=[])
                        nop.engine = ins.engine
                        nop.sync_info = mybir.SyncInfo(on_wait=[w], on_update=[])
                        out.append(nop)
                    ins.sync_info = mybir.SyncInfo(
                        on_wait=keep, on_update=list(si.on_update)
                    )
                    changed = True
                out.append(ins)
            if changed:
                bb.instructions = out


def _build_nc():
    import concourse.bass as bass
    import concourse.mybir as mybir
    import concourse.tile as tile
    from concourse.vector_clock import ScopedClock, VectorClock

    class LeanTailTileContext(tile.TileContext):
        """Tile's stock tail is drain + two all-engine barriers + sem clears;
        with the single-wait-per-instruction legalization the barrier waits
        explode into a ~10us serial EVSEM parade. Replace with: SP drain
        (single-wait NOPs), a two-semaphore handshake barrier (one wait per
        engine), then gpsimd range-clears everything last."""

        def _drain_and_barrier(self, tick_clock, wait_clock):
            nc = self.nc
            vc = tick_clock.global_clock
            n = len(vc)
            for proc in range(n):
                t = vc[proc]
                if t > 0:
                    nop = nc.sync.nop(nofuse=True, hint=f"tail_wait_{proc}")
                    req = ScopedClock(
                        {None: VectorClock([t if i == proc else 0 for i in range(n)])}
                    )
                    wait_clock.add_sem_waits(nop.ins, req)
            nc.sync.drain()

            semB = nc.alloc_semaphore("tail_barrier_b")
            semC = nc.alloc_semaphore("tail_barrier_c")
            engines = list(nc.engines.values())
            pool_eng = nc.gpsimd
            n_eng = len(engines)
            for e in engines:
                e.nop(nofuse=True, hint="tailb_inc").then_inc(semB, 1)
            for e in engines:
                e.wait_ge(semB, n_eng)
            for e in engines:
                if e is not pool_eng:
                    e.nop(nofuse=True, hint="tailc_inc").then_inc(semC, 1)
            pool_eng.wait_ge(semC, n_eng - 1)

            assert self.sems is not None
            popped = self.nc._tile_sem_poison_stack.pop()
            assert popped is self._sem_poison
            nc.clear_and_free_semaphores(
                list(self.sems.allocated().values()) + [semB, semC]
            )

    dt = mybir.dt

    nc = bass.Bass()

    # inputs host-packed as [pack, P, lane, cols]: K-chunk k = pack*4+lane,
    # so each transfer reads 4-8KB contiguous per partition (vs 1-2KB rows
    # of a plain [DIM, R] layout -- doubles effective HWDGE throughput)
    NP = KCH // 4  # packs of 4 K-chunks
    xf = nc.dram_tensor("xf", [NP, P, 4, RH], dt.bfloat16, kind="ExternalInput")
    xbk = nc.dram_tensor("xbk", [NP, P, 4, RH], dt.bfloat16, kind="ExternalInput")
    yf = nc.dram_tensor("yf", [NP, P, 4, RH], dt.bfloat16, kind="ExternalInput")
    ybk = nc.dram_tensor("ybk", [NP, P, 4, RH], dt.bfloat16, kind="ExternalInput")
    wxp = nc.dram_tensor("wxp", [NP, P, 4, CLASSES], dt.bfloat16, kind="ExternalInput")
    wyp = nc.dram_tensor("wyp", [NP, P, 4, CLASSES], dt.bfloat16, kind="ExternalInput")
    bxb = nc.dram_tensor("bxb", [P, CLASSES], dt.bfloat16, kind="ExternalInput")
    byb = nc.dram_tensor("byb", [P, CLASSES], dt.bfloat16, kind="ExternalInput")

    # combined output: [:, 0] = alpha_a, [:, 1] = alpha_x, [:, 2] = alpha_y
    aall = nc.dram_tensor("alpha_all", [R, 3, CLASSES], dt.bfloat16, kind="ExternalOutput")
    aall3 = aall.rearrange("(t p) x c -> t p x c", p=P)

    # softplus(x) = ln(exp(x) + 1); the installed ACT tables have no direct
    # softplus, but exp and ln share one table set. Pre-activation values are
    # within +-4 so exp cannot overflow.
    EXP = mybir.ActivationFunctionType.Exp
    LN = mybir.ActivationFunctionType.Ln
    ADD = mybir.AluOpType.add
    MULT = mybir.AluOpType.mult

    with LeanTailTileContext(nc) as tc:
        with (
            tc.tile_pool(name="wpool", bufs=1) as wpool,
            tc.tile_pool(name="xpool", bufs=1) as xpool,
            tc.tile_pool(name="epool", bufs=1) as epool,
            tc.tile_pool(name="opool", bufs=1) as opool,
            tc.tile_pool(name="psum", bufs=1, space="PSUM") as ppool,
        ):
            # -- input DMAs ------------------------------------------------
            # All input transfers trigger from the SP sequencer (HWDGE) in
            # first-release order; gates below keep arrivals ~3 K-chunks
            # ahead of PE use without flooding the DMA engines.
            dmas = []  # (handle, phase, k) ; phase None => ungated

            def dma(t_ap, src, ph, k, eng=None):
                h = (eng or nc.sync).dma_start(t_ap, src)
                dmas.append((h, ph, k))
                return h

            xf_sb, xb_sb, yf_sb, yb_sb, wx_sb, wy_sb = [], [], [], [], [], []
            for pp in range(NP):
                wx_sb.append(wpool.tile([P, 4, CLASSES], dt.bfloat16, tag=f"wx{pp}", name=f"wx{pp}"))
                xf_sb.append(xpool.tile([P, 4, RH], dt.bfloat16, tag=f"xf{pp}", name=f"xf{pp}"))
                xb_sb.append(xpool.tile([P, 4, RH], dt.bfloat16, tag=f"xb{pp}", name=f"xb{pp}"))
                wy_sb.append(wpool.tile([P, 4, CLASSES], dt.bfloat16, tag=f"wy{pp}", name=f"wy{pp}"))
                yf_sb.append(xpool.tile([P, 4, RH], dt.bfloat16, tag=f"yf{pp}", name=f"yf{pp}"))
                yb_sb.append(xpool.tile([P, 4, RH], dt.bfloat16, tag=f"yb{pp}", name=f"yb{pp}"))

            bx_sb = wpool.tile([P, CLASSES], dt.bfloat16, tag="bx")
            by_sb = wpool.tile([P, CLASSES], dt.bfloat16, tag="by")

            # Sync (SP) queue: x front + wx, then bulk back halves during Y0,
            # then the per-row-tile output DMAs. Queue order == release order
            # (anchors non-decreasing), so no head-of-line blocking.
            # First-touch K-chunks transfer singly so the k=0 matmul gate is
            # only ~380KB.
            # A single HWDGE queue sustains only ~90-150 GB/s (descriptor
            # fetch serialization), so the X0-era streams alternate between
            # the SP and ACT queues, each queue emitted in release order
            # (anchors monotone => no head-of-line blocking). Gated triggers
            # on ACT may block later ACT work, but the X0 epilogue's
            # softplus chain is only needed by the Y0 epilogue ~25us later;
            # the PSUM spills that unblock Y0 matmuls run on DVE.
            # Pack 0 ships per-lane and pack 1 per-half so the k=0 gate is
            # only ~380KB; later packs ship whole (4-8KB runs).
            sp_q, act_q = [], []  # (release_key, dst, src)

            def enq(q, key, dst, src):
                q.append((key, dst, src))

            def split_pack(pp):
                # (lane_lo, lane_hi, gate_chunk) pieces for pack pp
                if pp == 0:
                    return [(l, l + 1) for l in range(4)]
                if pp == 1:
                    return [(0, 2), (2, 4)]
                return [(0, 4)]

            ximp = 0
            for pp in range(NP):
                for (lo, hi) in split_pack(pp):
                    k0 = 4 * pp + lo
                    key = (0, max(0, k0 - 4)) if k0 >= 2 else (None, 0)
                    w_dst, w_src = wx_sb[pp][:, lo:hi, :], wxp[pp][:, lo:hi, :]
                    x_dst, x_src = xf_sb[pp][:, lo:hi, :], xf[pp][:, lo:hi, :]
                    if ximp % 2 == 0:
                        enq(sp_q, key, w_dst, w_src); enq(act_q, key, x_dst, x_src)
                    else:
                        enq(sp_q, key, x_dst, x_src); enq(act_q, key, w_dst, w_src)
                    ximp += 1
            enq(sp_q, (0, 8), bx_sb[:], bxb[:])
            # y-side front stream straddles late X0 / early Y0
            for pp in range(NP):
                kw = (0, 4 * pp + 5) if pp < 3 else (1, 0)
                ka = (0, 4 * pp + 7) if pp < 3 else (1, 2)
                kw = (kw[0], min(kw[1], KCH - 1)); ka = (ka[0], min(ka[1], KCH - 1))
                w_dst, w_src = wy_sb[pp][:], wyp[pp][:]
                y_dst, y_src = yf_sb[pp][:], yf[pp][:]
                if pp % 2 == 0:
                    enq(sp_q, kw, w_dst, w_src); enq(act_q, ka, y_dst, y_src)
                else:
                    enq(act_q, kw, w_dst, w_src); enq(sp_q, ka, y_dst, y_src)
            enq(act_q, (1, 3), by_sb[:], byb[:])
            # bulk x/y rows 512:1024 (row tiles 4-7) during Y0, on SP
            for pp in range(NP):
                enq(sp_q, (1, min(4 * pp + 4, KCH - 1)), xb_sb[pp][:], xbk[pp][:])
                enq(sp_q, (1, min(4 * pp + 6, KCH - 1)), yb_sb[pp][:], ybk[pp][:])

            def _key(e):
                (ph, k), _, _ = e
                return (-1, 0) if ph is None else (ph, k)
            for q, eng in ((sp_q, nc.sync), (act_q, nc.scalar)):
                for (ph, k), dst, src in sorted(q, key=_key):
                    dma(dst, src, ph, k, eng=eng)

            def x_slice(k, r):  # lhsT for global row tile r, K-chunk k
                sb = xf_sb if r < 4 else xb_sb
                rr = r % 4
                return sb[k // 4][:, k % 4, rr * P:(rr + 1) * P]

            def y_slice(k, r):
                sb = yf_sb if r < 4 else yb_sb
                rr = r % 4
                return sb[k // 4][:, k % 4, rr * P:(rr + 1) * P]

            def wx_slice(k, hs):
                return wx_sb[k // 4][:, k % 4, hs]

            def wy_slice(k, hs):
                return wy_sb[k // 4][:, k % 4, hs]

            HS = [slice(0, NH), slice(NH, CLASSES)]

            # -- compute ---------------------------------------------------
            from concourse.tile_rust import add_dep_helper

            UNITS = [(0, 4), (4, 2), (6, 1), (7, 1)]
            mm_anchor = {}  # (phase_idx, k) -> last MM instruction

            _out_tiles = {}

            def out_tile(r):
                if r not in _out_tiles:
                    _out_tiles[r] = opool.tile(
                        [P, 3, CLASSES], dt.bfloat16, tag=f"out{r % 4}", name=f"out{r % 4}"
                    )
                return _out_tiles[r]

            phase_idx = 0
            for u, (r0, nrt) in enumerate(UNITS):
                # ---- X phase ----
                psx = [
                    [ppool.tile([P, NH], dt.float32, tag=f"ps{jj}_{h}", name=f"ps{jj}_{h}") for h in range(2)]
                    for jj in range(nrt)
                ]
                for k in range(KCH):
                    st, sp = k == 0, k == KCH - 1
                    for jj in range(nrt):
                        lhsT = x_slice(k, r0 + jj)
                        nc.tensor.matmul(psx[jj][0][:], lhsT, wx_slice(k, HS[0]), start=st, stop=sp)
                        mm = nc.tensor.matmul(psx[jj][1][:], lhsT, wx_slice(k, HS[1]), start=st, stop=sp)
                    mm_anchor[(phase_idx, k)] = mm.ins
                phase_idx += 1

                # X epilogue (overlaps the Y phase): bias, softplus, alpha_x
                # into the combined out tile, then w1 = sp1/C + 1 in place.
                t1 = []
                for jj in range(nrt):
                    r = r0 + jj
                    ot = out_tile(r)
                    t_ = epool.tile([P, CLASSES], dt.float32, tag=f"t1_{jj}")
                    nc.vector.tensor_tensor(t_[:, HS[0]], psx[jj][0][:], bx_sb[:, HS[0]], ADD)
                    nc.vector.tensor_tensor(t_[:, HS[1]], psx[jj][1][:], bx_sb[:, HS[1]], ADD)
                    for h in range(2):
                        hs = HS[h]
                        sp1 = t_[:, hs]
                        nc.scalar.activation(sp1, sp1, EXP)
                        nc.scalar.activation(sp1, sp1, LN, bias=1.0)
                        nc.vector.tensor_scalar_add(ot[:, 1, hs], sp1, 1.0)
                        nc.vector.tensor_scalar(sp1, sp1, 1.0 / CLASSES, 1.0, MULT, ADD)
                    t1.append(t_)

                # ---- Y phase ----
                if u == len(UNITS) - 1:
                    # Last row tile: four 250-col quarter groups on separate
                    # PSUM banks; outputs ship per plane as each is ready so
                    # the final drain only waits on a 125KB aa transfer.
                    r = r0
                    QN = CLASSES // 4
                    ot = out_tile(r)
                    nc.sync.dma_start(aall3[r][:, 1, :], ot[:, 1, :])  # alpha_x
                    t_ = epool.tile([P, CLASSES], dt.float32, tag="t2_0")
                    for q in range(4):
                        qs = slice(QN * q, QN * (q + 1))
                        psq = ppool.tile([P, QN], dt.float32, tag=f"ps{q // 2}_{q % 2}", name=f"psq{q}")
                        for k in range(KCH):
                            st, sp = k == 0, k == KCH - 1
                            nc.tensor.matmul(psq[:], y_slice(k, r), wy_slice(k, qs), start=st, stop=sp)
                        sp2 = t_[:, qs]
                        nc.vector.tensor_tensor(sp2, psq[:], by_sb[:, qs], ADD)
                        nc.scalar.activation(sp2, sp2, EXP)
                        nc.scalar.activation(sp2, sp2, LN, bias=1.0)
                        nc.scalar.add(ot[:, 2, qs], sp2, 1.0)
                        nc.sync.dma_start(aall3[r][:, 2, qs], ot[:, 2, qs])  # alpha_y
                        nc.vector.tensor_tensor(sp2, sp2, t1[0][:, qs], MULT)
                        nc.vector.tensor_tensor(ot[:, 0, qs], sp2, ot[:, 1, qs], ADD)
                        nc.sync.dma_start(aall3[r][:, 0, qs], ot[:, 0, qs])  # alpha_a
                    phase_idx += 1
                    continue

                psy = [
                    [ppool.tile([P, NH], dt.float32, tag=f"ps{jj}_{h}", name=f"psy{jj}_{h}") for h in range(2)]
                    for jj in range(nrt)
                ]
                for k in range(KCH):
                    st, sp = k == 0, k == KCH - 1
                    for jj in range(nrt):
                        lhsT = y_slice(k, r0 + jj)
                        nc.tensor.matmul(psy[jj][0][:], lhsT, wy_slice(k, HS[0]), start=st, stop=sp)
                        mm = nc.tensor.matmul(psy[jj][1][:], lhsT, wy_slice(k, HS[1]), start=st, stop=sp)
                    mm_anchor[(phase_idx, k)] = mm.ins
                phase_idx += 1

                # Y epilogue: softplus, alpha_y, alpha_a = sp2*w1 + alpha_x,
                # then one combined output DMA per row tile.
                for jj in range(nrt):
                    r = r0 + jj
                    ot = out_tile(r)
                    t_ = epool.tile([P, CLASSES], dt.float32, tag=f"t2_{jj % 2}")
                    nc.vector.tensor_tensor(t_[:, HS[0]], psy[jj][0][:], by_sb[:, HS[0]], ADD)
                    nc.vector.tensor_tensor(t_[:, HS[1]], psy[jj][1][:], by_sb[:, HS[1]], ADD)
                    for h in range(2):
                        hs = HS[h]
                        sp2 = t_[:, hs]
                        nc.scalar.activation(sp2, sp2, EXP)
                        nc.scalar.activation(sp2, sp2, LN, bias=1.0)
                        nc.scalar.add(ot[:, 2, hs], sp2, 1.0)
                        nc.vector.tensor_tensor(sp2, sp2, t1[jj][:, hs], MULT)
                        nc.vector.tensor_tensor(ot[:, 0, hs], sp2, ot[:, 1, hs], ADD)
                        if u == len(UNITS) - 2:
                            # penultimate unit: ship halves as they finish so
                            # the final drain never waits on a 1.5MB transfer
                            nc.sync.dma_start(aall3[r][:, :, hs], ot[:, :, hs])
                    if u != len(UNITS) - 2:
                        nc.sync.dma_start(aall3[r], ot[:])

            # -- DMA backpressure: gate transfers on PE progress -----------
            for h, ph, k in dmas:
                if ph is not None:
                    add_dep_helper(
                        h.ins, mm_anchor[(ph, max(0, min(k, KCH - 1)))], reason="ramp"
                    )

    _split_waits(nc)
    return nc


def kernel(x, y, Wx, bx, Wy, by):
    global LAST_RESULTS
    from concourse.bass_utils import run_bass_kernel_spmd

    if "nc" not in _CACHE:
        _CACHE["nc"] = _build_nc()
    nc = _CACHE["nc"]

    bf16 = ml_dtypes.bfloat16
    x = np.asarray(x, dtype=np.float32)
    y = np.asarray(y, dtype=np.float32)
    xb = x.astype(bf16)                       # [BATCH, DIM]
    yb = y.astype(bf16)
    wxT = np.ascontiguousarray(np.asarray(Wx, dtype=np.float32).astype(bf16).T)  # [DIM, CLASSES]
    wyT = np.ascontiguousarray(np.asarray(Wy, dtype=np.float32).astype(bf16).T)
    bxb = np.ascontiguousarray(
        np.broadcast_to(np.asarray(bx, dtype=np.float32).astype(bf16), (P, CLASSES))
    )
    byb = np.ascontiguousarray(
        np.broadcast_to(np.asarray(by, dtype=np.float32).astype(bf16), (P, CLASSES))
    )

    xTb = np.ascontiguousarray(xb.T)          # [DIM, BATCH]
    yTb = np.ascontiguousarray(yb.T)

    def pack(m):  # [DIM, cols] -> [NP, P, 4, cols] with K-chunk k = pack*4+lane
        c = m.shape[1]
        return np.ascontiguousarray(
            m.reshape(KCH // 4, 4, P, c).transpose(0, 2, 1, 3)
        )

    wxpk = pack(wxT)
    wypk = pack(wyT)

    in_maps = []
    for c in range(NCORES):
        rs = slice(c * R, (c + 1) * R)
        xcT = xTb[:, rs]
        ycT = yTb[:, rs]
        in_maps.append(
            {
                "xf": pack(xcT[:, 0:RH]),
                "xbk": pack(xcT[:, RH:R]),
                "yf": pack(ycT[:, 0:RH]),
                "ybk": pack(ycT[:, RH:R]),
                "wxp": wxpk,
                "wyp": wypk,
                "bxb": bxb,
                "byb": byb,
            }
        )

    res = run_bass_kernel_spmd(nc, in_maps, core_ids=list(range(NCORES)))
    LAST_RESULTS = res

    parts = [res.results[c]["alpha_all"] for c in range(NCORES)]
    aa = np.concatenate([p[:, 0] for p in parts], axis=0).astype(np.float32)
    ax = np.concatenate([p[:, 1] for p in parts], axis=0).astype(np.float32)
    ay = np.concatenate([p[:, 2] for p in parts], axis=0).astype(np.float32)
    return (aa, ax, ay)
